# revision 19
# baseline (speedup 1.0000x reference)
"""GAT attention kernel for 8 trn2 NeuronCores (Bass/Tile).

Math (restructured from the reference to avoid materializing h_j):
    wa1 = W @ a1, wa2 = W @ a2                      (device, once)
    s[n,k]  = x0[n]·wa1 + x[n,k]·wa2                (since h@a1 = x0@(W a1))
    e       = leaky_relu(s, 0.2)
    p       = exp(e) * adj                          (no max-sub: scores are small)
    att     = (p + EPS) / (sum_k p + 16*EPS)        (== uniform 1/16 when row fully masked,
                                                     matching reference softmax of all -9e15)
    xbar[n] = sum_k att[n,k] * x[n,k,:]
    out     = elu((xbar + x0) @ W)                  (since h_prime + h = (xbar + x0)@W)
    elu(z)  = relu(z) + min(exp(z)-1, 0)

Sharding: node dim N padded 50000 -> 50176 = 8 cores * 49 tiles * 128 rows.
Per 128-row tile the 2048 (n,k) pairs form 16 blocks of [128 nk-rows, 128 feat]
held as x_tile[:, b*128:(b+1)*128] (host pre-permutes x accordingly so the DMA
is a single contiguous 1MB transfer per tile).

Per tile:
  DVE : 16x tensor_tensor_reduce (scores dot products), si TTR, reciprocal,
        att = (p+eps)*RZ_rep (STT), attseg = SEGBIG*att (one broadcast TT)
  PE  : si scatter (C-matmul), Z = group-sum (SEG), RZ broadcast (E8),
        x0^T via identity matmul + 16 xbar matmuls (accumulate xbarT in PSUM),
        final (xbar+x0)^T.T @ W
  ACT : Lrelu, Exp, Z+eps copy, PSUM->SBUF copies, exp/relu of final
  GPS : adj int->float cast, p = exp*adj, elu tail
"""

import numpy as np

N, K, F = 50000, 16, 128
ALPHA = 0.2
NCORES = 8
TILE = 128
NTILES = 49
RPC = TILE * NTILES          # rows per core = 6272
BPT = K                      # nk-blocks per tile = 16
EPS = 1e-12

_NC_CACHE = {}


def _consts_np():
    p = np.arange(128)
    j8 = np.arange(8)
    b16 = np.arange(16)
    ident = np.eye(128, dtype=np.float32)
    # C[n, q] = 1 iff n%8 == q//16   (si scatter: out[q,b] = si[8b + q//16])
    Cm = (p[:, None] % 8 == p[None, :] // 16).astype(np.float32)
    # SEGBIG[q, 8b+j] = 1 iff j == q//16  (pattern repeats over b)
    segbig = (p[:, None] // 16 == (p[None, :] % 8)).astype(np.float32)
    # E8[j, q] = 1 iff q//16 == j (rows 8..127 zero; used as lhsT [8,128])
    e8 = ((p[:, None] < 8) & (p[None, :] // 16 == p[:, None])).astype(np.float32)
    # SEG[q, j] = 1 iff q//16 == j   [128, 8]
    seg = (p[:, None] // 16 == j8[None, :]).astype(np.float32)
    # SEG8[n, b] = 1 iff n//8 == b   [128, 16]
    seg8 = (p[:, None] // 8 == b16[None, :]).astype(np.float32)
    ones = np.ones((128, 128), dtype=np.float32)
    return np.concatenate([ident, Cm, segbig, e8, seg, seg8, ones], axis=1)  # [128, 664]


def _consts_full_np(W, a):
    # consts + W + a1 + a2 packed into one tensor -> one setup DMA -> PE
    # matmuls see a single DMA semaphore lane (walrus allows only one sync
    # wait on an fp32 Matmult).
    return np.ascontiguousarray(
        np.concatenate(
            [_consts_np(), W.astype(np.float32),
             a[:F].astype(np.float32), a[F:].astype(np.float32)], axis=1)
    )  # [128, 794]


def _build_nc(ntiles=NTILES, finalize=True):
    import concourse.mybir as mybir
    import concourse.tile as tile
    from concourse import bacc

    fp = mybir.dt.float32
    i32 = mybir.dt.int32
    AF = mybir.ActivationFunctionType
    OP = mybir.AluOpType

    nc = bacc.Bacc("TRN2")
    # x tile data with x0 packed in the last 128 columns: [.., 2048 + 128]
    xd = nc.dram_tensor("xd", [ntiles, 128, BPT * F + F], fp, kind="ExternalInput")
    adjd = nc.dram_tensor("adjd", [ntiles, 128, K], i32, kind="ExternalInput")
    cst = nc.dram_tensor("cst", [128, 794], fp, kind="ExternalInput")
    yd = nc.dram_tensor("yd", [ntiles, 128, F], fp, kind="ExternalOutput")

    with tile.TileContext(nc) as tc:
        with (
            tc.tile_pool(name="const", bufs=1) as constp,
            tc.tile_pool(name="xin", bufs=3) as xin,
            tc.tile_pool(name="adjin", bufs=3) as adjin,
            tc.tile_pool(name="small", bufs=3) as small,
            tc.tile_pool(name="big", bufs=2) as big,
            tc.tile_pool(name="yout", bufs=3) as yout,
            tc.tile_pool(name="ps_mm", bufs=2, space="PSUM") as ps_mm,
            tc.tile_pool(name="ps_out", bufs=2, space="PSUM") as ps_out,
            tc.tile_pool(name="ps_sm", bufs=2, space="PSUM") as ps_sm,
        ):
            # ---------------- setup (single DMA -> single wait chains) ----
            consts = constp.tile([128, 794], fp)
            nc.sync.dma_start(out=consts, in_=cst[:, :])
            IDENT = consts[:, 0:128]
            Cm = consts[:, 128:256]
            SEGBIG = consts[:, 256:384]
            E8 = consts[:, 384:512]
            SEG = consts[:, 512:520]
            SEG8 = consts[:, 520:536]
            ONES = consts[:, 536:664]
            W_sb = consts[:, 664:792]
            a1_sb = consts[:, 792:793]
            a2_sb = consts[:, 793:794]

            # W^T via identity matmul
            WT_ps = ps_mm.tile([128, 128], fp, tag="xbarT")
            nc.tensor.matmul(WT_ps, lhsT=W_sb, rhs=IDENT, start=True, stop=True)
            WT_sb = constp.tile([128, 128], fp)
            nc.scalar.activation(out=WT_sb, in_=WT_ps, func=AF.Copy)

            # wa1 = W@a1, wa2 = W@a2 as columns
            wa_ps = ps_sm.tile([128, 2], fp, tag="si")
            nc.tensor.matmul(wa_ps[:, 0:1], lhsT=WT_sb, rhs=a1_sb, start=True, stop=True)
            nc.tensor.matmul(wa_ps[:, 1:2], lhsT=WT_sb, rhs=a2_sb, start=True, stop=True)
            wa_cols = constp.tile([128, 2], fp)
            nc.scalar.activation(out=wa_cols, in_=wa_ps, func=AF.Copy)

            # rows [1,128] = wa^T, then broadcast each row to 128 partitions
            wa_rep = {}
            for i in (0, 1):
                row_ps = ps_sm.tile([1, 128], fp, tag="zz")
                nc.tensor.matmul(row_ps, lhsT=wa_cols[:, i:i + 1], rhs=IDENT,
                                 start=True, stop=True)
                row_sb = constp.tile([1, 128], fp, tag=f"warow{i}")
                nc.scalar.activation(out=row_sb, in_=row_ps, func=AF.Copy)
                rep_ps = ps_out.tile([128, 128], fp, tag="zfin")
                nc.tensor.matmul(rep_ps, lhsT=ONES[0:1, :], rhs=row_sb[0:1, :],
                                 start=True, stop=True)
                rep_sb = constp.tile([128, 128], fp, tag=f"warep{i}")
                nc.scalar.activation(out=rep_sb, in_=rep_ps, func=AF.Copy)
                wa_rep[i] = rep_sb
            wa1_rep, wa2_rep = wa_rep[0], wa_rep[1]

            # ---------------- per-tile loop ----------------
            for t in range(ntiles):
                xall = xin.tile([128, BPT * F + F], fp, tag="x")
                nc.sync.dma_start(out=xall, in_=xd[t])
                x_tile = xall[:, 0:BPT * F]
                x0_tile = xall[:, BPT * F:BPT * F + F]
                adj_i = adjin.tile([128, K], i32, tag="adj")
                nc.sync.dma_start(out=adj_i, in_=adjd[t])
                adj_f = adjin.tile([128, K], fp, tag="adjf")
                nc.gpsimd.tensor_copy(out=adj_f, in_=adj_i)

                # self score si = x0 . wa1 (per row)
                scr = big.tile([128, 128], fp, tag="scr")
                si_nat = small.tile([128, 1], fp, tag="si_nat")
                nc.vector.scalar_tensor_tensor(
                    out=scr, in0=x0_tile, scalar=1.0, in1=wa1_rep,
                    op0=OP.mult, op1=OP.mult, accum_out=si_nat,
                )
                # scatter si into s-layout: si_s[q, b] = si[8b + q//16]
                Dt = small.tile([128, K], fp, tag="D")
                nc.gpsimd.tensor_scalar_mul(out=Dt, in0=SEG8, scalar1=si_nat)
                si_ps = ps_sm.tile([128, K], fp, tag="si")
                nc.tensor.matmul(si_ps, lhsT=Cm, rhs=Dt, start=True, stop=True)
                si_s = small.tile([128, K], fp, tag="si_s")
                nc.scalar.activation(out=si_s, in_=si_ps, func=AF.Copy)

                # neighbor scores: s[q, b] = x_blk_b[q,:] . wa2
                s = small.tile([128, K], fp, tag="s")
                for b in range(BPT):
                    nc.vector.scalar_tensor_tensor(
                        out=scr,
                        in0=x_tile[:, b * F:(b + 1) * F],
                        scalar=1.0,
                        in1=wa2_rep,
                        op0=OP.mult,
                        op1=OP.mult,
                        accum_out=s[:, b:b + 1],
                    )
                # s2 = s + si_s (gpsimd keeps DVE free)
                s2 = small.tile([128, K], fp, tag="s2")
                nc.gpsimd.tensor_add(out=s2, in0=s, in1=si_s)

                # leaky relu + exp + mask
                ls = small.tile([128, K], fp, tag="ls")
                nc.vector.scalar_tensor_tensor(
                    out=ls, in0=s2, scalar=ALPHA, in1=s2, op0=OP.mult, op1=OP.max,
                )
                exp_s = small.tile([128, K], fp, tag="exp_s")
                nc.scalar.activation(out=exp_s, in_=ls, func=AF.Exp)
                p_s = small.tile([128, K], fp, tag="p_s")
                nc.gpsimd.tensor_mul(out=p_s, in0=exp_s, in1=adj_f)

                # Z[j, b] = sum over partition-group j of p_s[:, b]
                Z_ps = ps_sm.tile([8, K], fp, tag="zz")
                nc.tensor.matmul(Z_ps, lhsT=SEG, rhs=p_s, start=True, stop=True)
                tz = small.tile([8, K], fp, tag="tz")
                nc.scalar.activation(out=tz, in_=Z_ps, func=AF.Copy, bias=16.0 * EPS)
                RZ = small.tile([8, K], fp, tag="RZ")
                nc.vector.reciprocal(RZ, tz)
                RZrep_ps = ps_sm.tile([128, K], fp, tag="zz")
                nc.tensor.matmul(RZrep_ps, lhsT=E8[0:8, :], rhs=RZ, start=True, stop=True)

                # att = (p + eps) * RZ_rep
                att = small.tile([128, K], fp, tag="att")
                nc.vector.scalar_tensor_tensor(
                    out=att, in0=p_s, scalar=EPS, in1=RZrep_ps,
                    op0=OP.add, op1=OP.mult,
                )

                # attseg[q, 8b+j] = SEGBIG[q, 8b+j] * att[q, b]
                attseg = big.tile([128, 128], fp, tag="attseg")
                att_bc = att.rearrange("p (b o) -> p b o", o=1).to_broadcast([128, K, 8])
                nc.vector.tensor_mul(
                    out=attseg.rearrange("p (b j) -> p b j", j=8),
                    in0=SEGBIG.rearrange("p (b j) -> p b j", j=8),
                    in1=att_bc,
                )

                # xbarT[f, n] accumulation: x0^T then per-block att-weighted sums
                xbarT_ps = ps_mm.tile([128, 128], fp, tag="xbarT")
                nc.tensor.matmul(xbarT_ps, lhsT=x0_tile, rhs=IDENT,
                                 start=True, stop=False, skip_group_check=True)
                for b in range(BPT):
                    nc.tensor.matmul(
                        xbarT_ps[:, 8 * b:8 * b + 8],
                        lhsT=x_tile[:, b * F:(b + 1) * F],
                        rhs=attseg[:, 8 * b:8 * b + 8],
                        start=False,
                        stop=(b == BPT - 1),
                        skip_group_check=True,
                    )
                ST_sb = big.tile([128, 128], fp, tag="ST")
                nc.scalar.activation(out=ST_sb, in_=xbarT_ps, func=AF.Copy)

                # final projection z = (xbar + x0) @ W  -> [128 n, 128 fo]
                zfin_ps = ps_out.tile([128, 128], fp, tag="zfin")
                nc.tensor.matmul(zfin_ps, lhsT=ST_sb, rhs=W_sb, start=True, stop=True)

                # elu(z) = relu(z) + min(exp(z)-1, 0)
                e_sb = big.tile([128, 128], fp, tag="e")
                nc.scalar.activation(out=e_sb, in_=zfin_ps, func=AF.Exp)
                r_sb = big.tile([128, 128], fp, tag="r")
                nc.scalar.activation(out=r_sb, in_=zfin_ps, func=AF.Relu)
                u_sb = big.tile([128, 128], fp, tag="u")
                nc.gpsimd.tensor_scalar(
                    out=u_sb, in0=e_sb, scalar1=1.0, scalar2=0.0,
                    op0=OP.subtract, op1=OP.min,
                )
                y_sb = yout.tile([128, 128], fp, tag="y")
                nc.gpsimd.tensor_add(out=y_sb, in0=r_sb, in1=u_sb)
                nc.sync.dma_start(out=yd[t], in_=y_sb)

    if finalize:
        nc.finalize()
    return nc


def _get_nc(ntiles=NTILES):
    if ntiles not in _NC_CACHE:
        _NC_CACHE[ntiles] = _build_nc(ntiles)
    return _NC_CACHE[ntiles]


def _shard_inputs(orignal_x, x, adj, W, a, ncores=NCORES, ntiles=NTILES):
    f32 = np.float32
    rpc = TILE * ntiles
    n_used = rpc * ncores
    x = np.asarray(x, f32)
    x0 = np.asarray(orignal_x, f32)
    adj = np.asarray(adj, np.int32)
    consts = _consts_full_np(np.asarray(W, f32), np.asarray(a, f32))
    n = x.shape[0]

    in_maps = []
    for c in range(ncores):
        lo = c * rpc
        hi = min((c + 1) * rpc, n)
        rows = hi - lo
        xc = x[lo:hi]
        x0c = x0[lo:hi]
        adjc = adj[lo:hi]
        if rows < rpc:
            pad = rpc - rows
            xc = np.concatenate([xc, np.zeros((pad, K, F), f32)])
            x0c = np.concatenate([x0c, np.zeros((pad, F), f32)])
            adjc = np.concatenate([adjc, np.zeros((pad, K), np.int32)])
        # per-tile layout [t, p, b*F+f] with x0 packed as trailing F columns
        xdev = np.empty((ntiles, 128, BPT * F + F), f32)
        xdev[:, :, :BPT * F] = xc.reshape(ntiles, BPT, 128, F).transpose(
            0, 2, 1, 3).reshape(ntiles, 128, BPT * F)
        xdev[:, :, BPT * F:] = x0c.reshape(ntiles, 128, F)
        adjdev = np.ascontiguousarray(adjc.reshape(ntiles, BPT, 128).transpose(0, 2, 1))
        in_maps.append({
            "xd": xdev,
            "adjd": adjdev,
            "cst": consts,
        })
    assert n <= n_used
    return in_maps


_LAST_RESULTS = None


def kernel(orignal_x, x, adj, W, a):
    import os
    os.environ.setdefault("JAX_PLATFORMS", "")
    from concourse.bass_utils import run_bass_kernel_spmd

    global _LAST_RESULTS
    nc = _get_nc()
    in_maps = _shard_inputs(orignal_x, x, adj, W, a)
    res = run_bass_kernel_spmd(nc, in_maps, list(range(NCORES)))
    _LAST_RESULTS = res
    y = np.concatenate([r["yd"].reshape(RPC, F) for r in res.results], axis=0)
    return np.ascontiguousarray(y[:N])
